# revision 34
# baseline (speedup 1.0000x reference)
"""CFConv (SchNet continuous-filter convolution) on 8 TRN2 NeuronCores.

Reference computation:
    f    = x @ W_in                       # (20000, 128)
    f_j  = f[idx_j]                       # (640000, 128) gather
    wf   = w_ij * f_j                     # elementwise
    conv = segment_sum(wf, seg_i)         # (20000, 128), seg_i sorted
    out  = conv @ W_out + b_out

Distribution: seg_i is sorted, so atoms are split into 8 contiguous
ranges of 2560 (padded to 20480); each core gets the edges targeting its
atom range.  No collectives needed — each core owns its output rows.

Per-core device pipeline (matmuls bf16 x bf16 or bf16 x fp8, f32 PSUM):
  Phase A: f = x @ W_in computed locally (replicated), written to two
           internal HBM half-tables (bf16 rows, split so phase B can
           start after only the low half is ready).
  Phase B: edges processed in groups of 128 (one group = one matmul
           contraction).  Groups are host-packed per 128-atom window,
           split into lo/hi f-table halves, each padded to fixed group
           counts k_lo/k_hi so the graph is identical on all cores (the
           dma_gather descriptor generation on the Q7 cores is the
           kernel's throughput floor at ~2.2ns/idx).
    - w_ij group tiles DMA'd from HBM (host-reordered, bf16)
    - f_j rows fetched with gpsimd.dma_gather (MoE gather primitive),
      two calls per window striped across all 4 SWDGE queues so the
      four Q7 core pairs generate descriptors in parallel
    - wf = w * f_j on VectorE
    - segment-sum via TensorE: psum[fm, atom_window] += wf_g^T @ S_g
      where S_g is the host-built 0/1 edge->atom one-hot (fp8 rhs)
    - out^T = W_out^T @ conv^T (TensorE), bias via ScalarE, transposed
      back per 128x128 tile on TensorE, DMA'd to the output shard.

Atoms are host-relabeled (snake-deal by per-atom edge count) so every
window carries a near-equal edge count, minimizing the uniform padding;
the output is un-permuted on the host after the gather.

Measured on 8 axon TRN2 cores: ~309-330 us HW exec on quiet runs
(shared-HBM neighbor noise can add 10-20%), rel err 5.2e-3 vs the f32
reference.
"""

import numpy as np
import ml_dtypes

import concourse.bacc as bacc
import concourse.bass as bass
import concourse.mybir as mybir
import concourse.tile as tile
from concourse.bass_utils import run_bass_kernel_spmd

BF16 = ml_dtypes.bfloat16
FP8 = ml_dtypes.float8_e4m3

N_ATOMS = 20000
N_EDGES = 640000
F = 128
N_CORES = 8
A_CORE = 2560                 # padded atoms per core
A_PAD = A_CORE * N_CORES      # 20480
CHUNK = 512                   # atoms per PSUM chunk (one bank)
WIN = 128                     # atoms per window (matmul N dim)
WIN_PER_CORE = A_CORE // WIN  # 20
N_WIN = A_PAD // WIN          # 160
WB = 2                        # windows per gather-call batch

TRACE = False                 # set True (with ntff shim) for profiling
_BUILD_CACHE: dict = {}


def _build(k_lo: int, k_hi: int):
    """Build the SPMD Bass graph for given per-window group counts.

    Each 128-atom window's edges are split into a lo half (f rows
    [0, A_PAD/2)) and a hi half, each padded to k_lo/k_hi groups of 128;
    the two dma_gathers per window depend only on their half of the f
    table, so phase B overlaps the tail of phase A.
    """
    key = (k_lo, k_hi)
    if key in _BUILD_CACHE:
        return _BUILD_CACHE[key]

    k_fix = k_lo + k_hi
    G = WIN_PER_CORE * k_fix      # groups per core
    E = G * 128                   # padded edges per core
    H = A_PAD if k_hi == 0 else A_PAD // 2
    bf = mybir.dt.bfloat16
    f32 = mybir.dt.float32

    nc = bacc.Bacc("TRN2", target_bir_lowering=False, debug=False,
                   num_swdge_queues=4, num_devices=N_CORES)
    xT_e = nc.dram_tensor("xT", [128, A_PAD], bf, kind="ExternalInput")
    w_in_e = nc.dram_tensor("w_in", [128, 128], bf, kind="ExternalInput")
    w_out_e = nc.dram_tensor("w_out", [128, 128], bf, kind="ExternalInput")
    b_e = nc.dram_tensor("b_out", [128, 1], f32, kind="ExternalInput")
    id_e = nc.dram_tensor("ident", [128, 128], bf, kind="ExternalInput")
    w_ed_e = nc.dram_tensor("w_ed", [128, G, F], bf, kind="ExternalInput")
    rel_e = nc.dram_tensor("relw", [128, G], bf, kind="ExternalInput")
    iota_e = nc.dram_tensor("iotaf", [128, WIN], bf, kind="ExternalInput")
    idx_e = nc.dram_tensor("idxw", [128, E // 16], mybir.dt.int16,
                           kind="ExternalInput")
    out_e = nc.dram_tensor("out", [A_CORE, F], f32, kind="ExternalOutput")

    with tile.TileContext(nc) as tc:
        with (
            tc.tile_pool(name="dram", bufs=1, space="DRAM") as dpool,
            tc.tile_pool(name="const", bufs=1) as cpool,
            tc.tile_pool(name="pha", bufs=3) as apool,
            tc.tile_pool(name="psA", bufs=3, space="PSUM") as psA,
        ):
            f_lo_hbm = dpool.tile([H, F], bf)
            f_hi_hbm = dpool.tile([H, F], bf, name="f_hi_hbm") if k_hi else None

            w_in_t = cpool.tile([128, 128], bf)
            nc.sync.dma_start(w_in_t[:], w_in_e[:])
            w_out_t = cpool.tile([128, 128], bf)
            nc.sync.dma_start(w_out_t[:], w_out_e[:])
            b_t = cpool.tile([128, 1], f32)
            nc.sync.dma_start(b_t[:], b_e[:])
            id_t = cpool.tile([128, 128], bf)
            nc.sync.dma_start(id_t[:], id_e[:])
            iota_t = cpool.tile([128, 1, WIN], bf)
            nc.sync.dma_start(iota_t[:, 0, :], iota_e[:])
            # idx issues first on the scalar HWDGE queue (it gates the
            # first gather); then the xT chunks — 8 small ones so the
            # matmul pipeline starts as soon as the first 2560 atoms land.
            idx_t = cpool.tile([128, E // 16], mybir.dt.int16)
            nc.scalar.dma_start(idx_t[:], idx_e[:])

            # ---------------- Phase A: f table ----------------
            QW = A_PAD // 8
            f_sb = None
            for x4 in range(8):
                xq_t = apool.tile([128, QW], bf, tag="xq")
                nc.scalar.dma_start(xq_t[:], xT_e[:, x4 * QW:(x4 + 1) * QW])
                for t4q in range(QW // 512):
                    t4 = x4 * (QW // 512) + t4q
                    ps = psA.tile([128, 4, 128], f32)
                    for q in range(4):
                        tl = t4q * 4 + q
                        nc.tensor.matmul(
                            ps[:, q, :],
                            xq_t[:, tl * 128:(tl + 1) * 128],
                            w_in_t[:],
                            start=True, stop=True,
                        )
                    j = t4 % 2
                    if j == 0:
                        f_sb = apool.tile([128, 8, F], bf, tag="fsb")
                    # alternate the psum->bf16 casts between DVE and ACT so
                    # neither engine's other work (IS_EQ one-hots on DVE,
                    # DMA issue on ACT) can stall the f-table chain
                    if t4 % 2 == 0:
                        nc.vector.tensor_copy(
                            f_sb[:, j * 4:(j + 1) * 4, :], ps[:])
                    else:
                        nc.scalar.copy(
                            f_sb[:, j * 4:(j + 1) * 4, :], ps[:])
                    if j == 1:
                        a0 = (t4 - 1) * 512
                        tgt = f_lo_hbm if a0 < H else f_hi_hbm
                        a0 = a0 % H
                        dst = tgt[a0:a0 + 1024, :].rearrange(
                            "(j p) f -> p j f", p=128)
                        nc.sync.dma_start(dst, f_sb[:])

            # rel lands only after the xT chunks: its consumers (the IS_EQ
            # one-hot builders on DVE) must not be schedulable during phase
            # A, where they would displace the psum->f casts from the DVE
            # and stretch the f-table critical path.
            rel_t = cpool.tile([128, G, 1], bf)
            nc.scalar.dma_start(rel_t[:, :, 0], rel_e[:])

            # ---------------- Phase B: edges ----------------
            # Gathers batched over WB windows per call pair (lo + hi): slot
            # layout per batch is [w0.lo .. w_{WB-1}.lo | w0.hi .. w_{WB-1}.hi]
            # (host-matched).  Fewer calls amortize the ~2us fixed cost each
            # dma_gather pays on the GpSimd engine.
            kb_lo = WB * k_lo
            kb_hi = WB * k_hi
            kb_fix = kb_lo + kb_hi
            NB = WIN_PER_CORE // WB
            with (
                tc.tile_pool(name="wp", bufs=1) as wpool,
                tc.tile_pool(name="phb", bufs=2) as bpool,
                tc.tile_pool(name="fjp", bufs=4) as fjpool,
                tc.tile_pool(name="psC", bufs=2, space="PSUM") as pscp,
                tc.tile_pool(name="ps2", bufs=2, space="PSUM") as ps2p,
                tc.tile_pool(name="ps3", bufs=1, space="PSUM") as ps3p,
            ):
                state = {}
                psc_box = [None]

                def issue_front(bt):
                    # w on a bufs=1 pool: batch bt's load issues only after
                    # batch bt-1's scatter released the buffer, keeping the
                    # lead-in DMA queues clear for phase A + the f_lo writes.
                    w_t = wpool.tile([128, kb_fix, F], bf, tag="w")
                    nc.scalar.dma_start(
                        w_t[:], w_ed_e[:, bt * kb_fix:(bt + 1) * kb_fix, :])
                    base8 = bt * kb_fix * 8
                    fj_t = fjpool.tile([128, kb_fix, F], bf, tag="fj")
                    nc.gpsimd.dma_gather(
                        fj_t[:, 0:kb_lo, :], f_lo_hbm[:, :],
                        idx_t[:, base8:base8 + kb_lo * 8],
                        num_idxs=kb_lo * 128,
                        num_idxs_reg=kb_lo * 128,
                        elem_size=F,
                        single_packet=False,
                        queue_num=(2 * bt if k_hi else bt) % 4,
                    )
                    state[bt] = (w_t, fj_t)

                def issue_back(bt):
                    w_t, fj_t = state.pop(bt)
                    # scatter one-hots generated on DVE (saves 10.8MB of DMA
                    # that would otherwise compete with the gather drains);
                    # emitted here so the DVE FIFO orders them after the
                    # phase-A casts: S[p, g, c] = (rel[p, g] == c)
                    s_t = bpool.tile([128, kb_fix, WIN], mybir.dt.float8e4,
                                     tag="s")
                    nc.vector.tensor_tensor(
                        s_t[:],
                        rel_t[:, bt * kb_fix:(bt + 1) * kb_fix, :]
                        .to_broadcast([128, kb_fix, WIN]),
                        iota_t[:].to_broadcast([128, kb_fix, WIN]),
                        mybir.AluOpType.is_equal)
                    base8 = bt * kb_fix * 8
                    if k_hi:
                        nc.gpsimd.dma_gather(
                            fj_t[:, kb_lo:kb_fix, :], f_hi_hbm[:, :],
                            idx_t[:, base8 + kb_lo * 8:base8 + kb_fix * 8],
                            num_idxs=kb_hi * 128,
                            num_idxs_reg=kb_hi * 128,
                            elem_size=F,
                            single_packet=False,
                            queue_num=(2 * bt + 1) % 4,
                        )
                    wf_t = bpool.tile([128, kb_fix, F], bf, tag="wf")
                    # one multiply per window-section so each window's
                    # scatter can start as soon as its own slots are ready
                    for wi in range(WB):
                        lo0, lo1 = wi * k_lo, (wi + 1) * k_lo
                        nc.vector.tensor_tensor(
                            wf_t[:, lo0:lo1, :], w_t[:, lo0:lo1, :],
                            fj_t[:, lo0:lo1, :], mybir.AluOpType.mult)
                        hi0 = kb_lo + wi * k_hi
                        hi1 = kb_lo + (wi + 1) * k_hi
                        nc.vector.tensor_tensor(
                            wf_t[:, hi0:hi1, :], w_t[:, hi0:hi1, :],
                            fj_t[:, hi0:hi1, :], mybir.AluOpType.mult)

                    for wi in range(WB):
                        wk = bt * WB + wi
                        ch = wk // 4
                        col = WIN * (wk % 4)
                        if wk % 4 == 0:
                            psc = pscp.tile([128, CHUNK], f32)
                            psc_box[0] = psc
                        psc = psc_box[0]
                        for g in range(k_fix):
                            sl = (wi * k_lo + g if g < k_lo
                                  else kb_lo + wi * k_hi + (g - k_lo))
                            nc.tensor.matmul(
                                psc[:, col:col + WIN],
                                wf_t[:, sl, :],
                                s_t[:, sl, :],
                                start=(g == 0), stop=(g == k_fix - 1),
                            )

                        if wk % 4 == 3:
                            convT = bpool.tile([128, CHUNK], bf, tag="convT")
                            nc.vector.tensor_copy(convT[:], psc[:])
                            ps2 = ps2p.tile([128, CHUNK], f32)
                            nc.tensor.matmul(ps2[:], w_out_t[:], convT[:],
                                             start=True, stop=True)
                            outT = bpool.tile([128, CHUNK], bf, tag="outT")
                            nc.scalar.activation(
                                outT[:], ps2[:],
                                mybir.ActivationFunctionType.Identity,
                                bias=b_t[:],
                            )
                            outf = bpool.tile([128, 4, F], f32, tag="outf")
                            for t in range(4):
                                ps3 = ps3p.tile([128, 128], bf)
                                nc.tensor.transpose(
                                    ps3[:], outT[:, t * 128:(t + 1) * 128],
                                    id_t[:])
                                nc.vector.tensor_copy(outf[:, t, :], ps3[:])
                            dst = out_e[ch * CHUNK:(ch + 1) * CHUNK,
                                        :].rearrange("(t p) f -> p t f", p=128)
                            nc.sync.dma_start(dst, outf[:])

                # lo-gathers lead the hi-gathers by one batch: the lo calls
                # for batches 0 and 1 fill the window where f_hi is still
                # being written by phase A.
                issue_front(0)
                for bt in range(1, NB):
                    issue_front(bt)
                    issue_back(bt - 1)
                issue_back(NB - 1)

    nc.compile()
    _BUILD_CACHE[key] = nc
    return nc


def _prep(x, w_ij, seg_i, idx_j, W_in, W_out, b_out):
    """Host-side sharding: reorder/pad edges, build S one-hots, wrap idxs."""
    x = np.asarray(x, dtype=np.float32)
    w_ij = np.asarray(w_ij, dtype=np.float32)
    seg = np.asarray(seg_i).astype(np.int64)
    idxj = np.asarray(idx_j).astype(np.int64)

    # Relabel atoms so every 128-atom window gets a near-equal edge count
    # (snake-deal atoms in decreasing edge-count order over the windows).
    # This minimizes the uniform per-window padding k_lo/k_hi, which sets
    # the dma_gather descriptor-generation floor.  seg/idx/x/f-table/output
    # all permute consistently; the output is un-permuted on the host.
    cnt = np.bincount(seg, minlength=N_ATOMS)
    order = np.argsort(-cnt, kind="stable")
    i = np.arange(N_ATOMS)
    r, c = np.divmod(i, N_WIN)
    w = np.where(r % 2 == 0, c, N_WIN - 1 - c)
    perm = np.empty(N_ATOMS, np.int64)
    perm[order] = w * WIN + r
    seg = perm[seg]
    idxj = perm[idxj]
    o = np.argsort(seg, kind="stable")
    seg, idxj, w_ij = seg[o], idxj[o], w_ij[o]

    bounds = np.searchsorted(seg, np.arange(N_WIN + 1) * WIN)
    Hs = A_PAD // 2

    # per-window lo/hi split (f-table halves)
    lo_ids, hi_ids, lo_v, hi_v = [], [], [], []
    n_lo = np.zeros(N_WIN, np.int64)
    n_hi = np.zeros(N_WIN, np.int64)
    for k in range(N_WIN):
        b0, b1 = bounds[k], bounds[k + 1]
        ids = np.arange(b0, b1)
        v = idxj[b0:b1]
        m = v < Hs
        lo_ids.append(ids[m])
        hi_ids.append(ids[~m])
        lo_v.append(v[m].astype(np.int16))
        hi_v.append((v[~m] - Hs).astype(np.int16))
        n_lo[k] = m.sum()
        n_hi[k] = (~m).sum()
    k_lo = max(1, int(np.ceil(n_lo.max() / 128)))
    k_hi = max(1, int(np.ceil(n_hi.max() / 128)))
    k_fix = k_lo + k_hi
    e_win = k_fix * 128
    g_core = WIN_PER_CORE * k_fix
    e_pad = g_core * 128

    # padded edge-id + gather-idx matrices in lo|hi order
    eidx = np.zeros((N_WIN, e_win), np.int64)
    valid = np.zeros((N_WIN, e_win), bool)
    gidx = np.zeros((N_WIN, e_win), np.int16)
    for k in range(N_WIN):
        a, b = n_lo[k], n_hi[k]
        eidx[k, :a] = lo_ids[k]
        valid[k, :a] = True
        gidx[k, :a] = lo_v[k]
        off = k_lo * 128
        eidx[k, off:off + b] = hi_ids[k]
        valid[k, off:off + b] = True
        gidx[k, off:off + b] = hi_v[k]

    w_bf = w_ij.astype(BF16)

    xT = np.zeros((128, A_PAD), BF16)
    xT[:, perm] = np.ascontiguousarray(x.T).astype(BF16)
    shared = {
        "xT": xT,
        "w_in": np.asarray(W_in, np.float32).astype(BF16),
        "w_out": np.asarray(W_out, np.float32).astype(BF16),
        "b_out": np.asarray(b_out, np.float32).reshape(128, 1).copy(),
        "ident": np.eye(128, dtype=BF16),
        "iotaf": np.broadcast_to(
            np.arange(WIN, dtype=np.float32), (128, WIN)).astype(BF16),
    }

    # Slot-block order per core: per WB-window batch, all lo sections then
    # all hi sections (must match the device-side gather/scatter layout).
    n_lo_s, n_hi_s = k_lo * 128, k_hi * 128
    in_maps = []
    for c in range(N_CORES):
        w0 = c * WIN_PER_CORE
        ei_b, va_b, gi_lo, gi_hi, wbase = [], [], [], [], []
        for bt in range(WIN_PER_CORE // WB):
            for wi in range(WB):
                k = w0 + bt * WB + wi
                ei_b.append(eidx[k, :n_lo_s])
                va_b.append(valid[k, :n_lo_s])
                gi_lo.append(gidx[k, :n_lo_s])
                wbase.append(np.full(n_lo_s, k * WIN, np.int64))
            for wi in range(WB):
                k = w0 + bt * WB + wi
                ei_b.append(eidx[k, n_lo_s:])
                va_b.append(valid[k, n_lo_s:])
                gi_hi.append(gidx[k, n_lo_s:])
                wbase.append(np.full(n_hi_s, k * WIN, np.int64))
        ei = np.concatenate(ei_b)
        va = np.concatenate(va_b)
        wb_s = np.concatenate(wbase)

        w_rows = np.zeros((e_pad, F), BF16)
        w_rows[va] = w_bf[ei[va]]
        w_ed = np.ascontiguousarray(
            w_rows.reshape(g_core, 128, F).transpose(1, 0, 2))

        rel = np.where(va, seg[ei] - wb_s, -1)
        relw = np.ascontiguousarray(
            rel.reshape(g_core, 128).T.astype(BF16))

        # wrapped idx layout, one wrap per gather call (lo and hi per batch)
        blocks = []
        for bt in range(WIN_PER_CORE // WB):
            lo_cat = np.concatenate(gi_lo[bt * WB:(bt + 1) * WB])
            blocks.append(lo_cat.reshape(-1, 16).T)
            if k_hi:
                hi_cat = np.concatenate(gi_hi[bt * WB:(bt + 1) * WB])
                blocks.append(hi_cat.reshape(-1, 16).T)
        idxw = np.ascontiguousarray(
            np.tile(np.concatenate(blocks, axis=1), (8, 1)))

        m = dict(shared)
        m["w_ed"] = w_ed
        m["relw"] = relw
        m["idxw"] = idxw
        in_maps.append(m)
    return k_lo, k_hi, in_maps, perm


def kernel(x, w_ij, seg_i, idx_j, seg_i_sum, W_in, W_out, b_out):
    k_lo, k_hi, in_maps, perm = _prep(x, w_ij, seg_i, idx_j, W_in, W_out,
                                      b_out)
    nc = _build(k_lo, k_hi)
    res = run_bass_kernel_spmd(nc, in_maps, core_ids=list(range(N_CORES)),
                               trace=TRACE)
    kernel.last_result = res
    out = np.concatenate(
        [np.asarray(res.results[c]["out"]) for c in range(N_CORES)], axis=0)
    return np.ascontiguousarray(out[perm]).astype(np.float32)



# revision 36
# speedup vs baseline: 1.1462x; 1.1462x over previous
"""CFConv (SchNet continuous-filter convolution) on 8 TRN2 NeuronCores.

Reference computation:
    f    = x @ W_in                       # (20000, 128)
    f_j  = f[idx_j]                       # (640000, 128) gather
    wf   = w_ij * f_j                     # elementwise
    conv = segment_sum(wf, seg_i)         # (20000, 128), seg_i sorted
    out  = conv @ W_out + b_out

Distribution: seg_i is sorted, so atoms are split into 8 contiguous
ranges of 2560 (padded to 20480); each core gets the edges targeting its
atom range.  No collectives needed — each core owns its output rows.

Per-core device pipeline (matmuls bf16 x bf16 or bf16 x fp8, f32 PSUM):
  Phase A: f = x @ W_in computed locally (replicated), written to two
           internal HBM half-tables (bf16 rows, split so phase B can
           start after only the low half is ready).  xT streams in 8
           small chunks (pipeline starts on the first 2560 atoms) and
           the psum->bf16 casts alternate between DVE and ACT so
           neither engine's other work stalls the f-table chain.
  Phase B: edges processed in groups of 128 (one group = one matmul
           contraction), host-packed per 128-atom window, batched WB=2
           windows per gather-call pair.  A dma_gather holds the Pool
           engine through its DMA drain (~2.4-2.8ns/idx observed), so
           the 20 calls are the serial throughput floor; everything
           else is arranged to keep that stream gapless:
    - lo-gathers lead hi-gathers by one batch (f_hi still writing)
    - idx DMA issues first on the scalar HWDGE queue, then the xT
      chunks; w tiles use a bufs=1 pool so only one batch prefetches,
      keeping lead-in bandwidth for the f-table writes the first
      gather waits on
    - scatter one-hots are generated on DVE (iota==rel broadcast
      is_equal -> fp8), saving 10.8MB/core of DMA; their rel input is
      DMA'd after the xT chunks so the IS_EQ ops cannot be scheduled
      into phase A
    - wf = w * f_j on VectorE
    - segment-sum via TensorE: psum[fm, atom_window] += wf_g^T @ S_g
    - out^T = W_out^T @ conv^T (TensorE), bias via ScalarE, transposed
      back per 128x128 tile on TensorE, DMA'd to the output shard.

Atoms are host-relabeled (snake-deal by per-atom edge count) so every
window carries a near-equal edge count, minimizing the uniform padding;
the output is un-permuted on the host after the gather.

Found not to work: prepare_only+trigger_dma (SWDGE ring blocks, Tile
consumer sync broken), indirect_dma_start (7x slower + wrong data on
HW), single merged f table (a call drains through one SWDGE queue's
engines only - the lo/hi pair's 2-queue parallelism is load-bearing),
-1 padding indices (hangs the DMA completion sem accounting).

Measured on 8 axon TRN2 cores: ~332-343 us HW exec (vs ~399 us for
the 40-call baseline on the same session; run-to-run HBM-neighbor
noise is +-15%), rel err 5.2e-3 vs the f32 reference.
"""

import numpy as np
import ml_dtypes

import concourse.bacc as bacc
import concourse.bass as bass
import concourse.mybir as mybir
import concourse.tile as tile
from concourse.bass_utils import run_bass_kernel_spmd

BF16 = ml_dtypes.bfloat16
FP8 = ml_dtypes.float8_e4m3

N_ATOMS = 20000
N_EDGES = 640000
F = 128
N_CORES = 8
A_CORE = 2560                 # padded atoms per core
A_PAD = A_CORE * N_CORES      # 20480
CHUNK = 512                   # atoms per PSUM chunk (one bank)
WIN = 128                     # atoms per window (matmul N dim)
WIN_PER_CORE = A_CORE // WIN  # 20
N_WIN = A_PAD // WIN          # 160
WB = 2                        # windows per gather-call batch

TRACE = False                 # set True (with ntff shim) for profiling
_BUILD_CACHE: dict = {}


def _build(k_lo: int, k_hi: int):
    """Build the SPMD Bass graph for given per-window group counts.

    Each 128-atom window's edges are split into a lo half (f rows
    [0, A_PAD/2)) and a hi half, each padded to k_lo/k_hi groups of 128;
    the two dma_gathers per window depend only on their half of the f
    table, so phase B overlaps the tail of phase A.
    """
    key = (k_lo, k_hi)
    if key in _BUILD_CACHE:
        return _BUILD_CACHE[key]

    k_fix = k_lo + k_hi
    G = WIN_PER_CORE * k_fix      # groups per core
    E = G * 128                   # padded edges per core
    H = A_PAD if k_hi == 0 else A_PAD // 2
    bf = mybir.dt.bfloat16
    f32 = mybir.dt.float32

    nc = bacc.Bacc("TRN2", target_bir_lowering=False, debug=False,
                   num_swdge_queues=4, num_devices=N_CORES)
    xT_e = nc.dram_tensor("xT", [128, A_PAD], bf, kind="ExternalInput")
    w_in_e = nc.dram_tensor("w_in", [128, 128], bf, kind="ExternalInput")
    w_out_e = nc.dram_tensor("w_out", [128, 128], bf, kind="ExternalInput")
    b_e = nc.dram_tensor("b_out", [128, 1], f32, kind="ExternalInput")
    id_e = nc.dram_tensor("ident", [128, 128], bf, kind="ExternalInput")
    w_ed_e = nc.dram_tensor("w_ed", [128, G, F], bf, kind="ExternalInput")
    rel_e = nc.dram_tensor("relw", [128, G], bf, kind="ExternalInput")
    iota_e = nc.dram_tensor("iotaf", [128, WIN], bf, kind="ExternalInput")
    idx_e = nc.dram_tensor("idxw", [128, E // 16], mybir.dt.int16,
                           kind="ExternalInput")
    out_e = nc.dram_tensor("out", [A_CORE, F], f32, kind="ExternalOutput")

    with tile.TileContext(nc) as tc:
        with (
            tc.tile_pool(name="dram", bufs=1, space="DRAM") as dpool,
            tc.tile_pool(name="const", bufs=1) as cpool,
            tc.tile_pool(name="pha", bufs=3) as apool,
            tc.tile_pool(name="psA", bufs=3, space="PSUM") as psA,
        ):
            f_lo_hbm = dpool.tile([H, F], bf)
            f_hi_hbm = dpool.tile([H, F], bf, name="f_hi_hbm") if k_hi else None

            w_in_t = cpool.tile([128, 128], bf)
            nc.sync.dma_start(w_in_t[:], w_in_e[:])
            w_out_t = cpool.tile([128, 128], bf)
            nc.sync.dma_start(w_out_t[:], w_out_e[:])
            b_t = cpool.tile([128, 1], f32)
            nc.sync.dma_start(b_t[:], b_e[:])
            id_t = cpool.tile([128, 128], bf)
            nc.sync.dma_start(id_t[:], id_e[:])
            iota_t = cpool.tile([128, 1, WIN], bf)
            nc.sync.dma_start(iota_t[:, 0, :], iota_e[:])
            # idx issues first on the scalar HWDGE queue (it gates the
            # first gather); then the xT chunks — 8 small ones so the
            # matmul pipeline starts as soon as the first 2560 atoms land.
            idx_t = cpool.tile([128, E // 16], mybir.dt.int16)
            nc.scalar.dma_start(idx_t[:], idx_e[:])

            # ---------------- Phase A: f table ----------------
            QW = A_PAD // 8
            f_sb = None
            for x4 in range(8):
                xq_t = apool.tile([128, QW], bf, tag="xq")
                nc.scalar.dma_start(xq_t[:], xT_e[:, x4 * QW:(x4 + 1) * QW])
                for t4q in range(QW // 512):
                    t4 = x4 * (QW // 512) + t4q
                    ps = psA.tile([128, 4, 128], f32)
                    for q in range(4):
                        tl = t4q * 4 + q
                        nc.tensor.matmul(
                            ps[:, q, :],
                            xq_t[:, tl * 128:(tl + 1) * 128],
                            w_in_t[:],
                            start=True, stop=True,
                        )
                    j = t4 % 2
                    if j == 0:
                        f_sb = apool.tile([128, 8, F], bf, tag="fsb")
                    # alternate the psum->bf16 casts between DVE and ACT so
                    # neither engine's other work (IS_EQ one-hots on DVE,
                    # DMA issue on ACT) can stall the f-table chain
                    if t4 % 2 == 0:
                        nc.vector.tensor_copy(
                            f_sb[:, j * 4:(j + 1) * 4, :], ps[:])
                    else:
                        nc.scalar.copy(
                            f_sb[:, j * 4:(j + 1) * 4, :], ps[:])
                    if j == 1:
                        a0 = (t4 - 1) * 512
                        tgt = f_lo_hbm if a0 < H else f_hi_hbm
                        a0 = a0 % H
                        dst = tgt[a0:a0 + 1024, :].rearrange(
                            "(j p) f -> p j f", p=128)
                        nc.sync.dma_start(dst, f_sb[:])

            # rel lands only after the xT chunks: its consumers (the IS_EQ
            # one-hot builders on DVE) must not be schedulable during phase
            # A, where they would displace the psum->f casts from the DVE
            # and stretch the f-table critical path.
            rel_t = cpool.tile([128, G, 1], bf)
            nc.scalar.dma_start(rel_t[:, :, 0], rel_e[:])

            # ---------------- Phase B: edges ----------------
            # Gathers batched over WB windows per call pair (lo + hi): slot
            # layout per batch is [w0.lo .. w_{WB-1}.lo | w0.hi .. w_{WB-1}.hi]
            # (host-matched).  Fewer calls amortize the ~2us fixed cost each
            # dma_gather pays on the GpSimd engine.
            kb_lo = WB * k_lo
            kb_hi = WB * k_hi
            kb_fix = kb_lo + kb_hi
            NB = WIN_PER_CORE // WB
            with (
                tc.tile_pool(name="wp", bufs=1) as wpool,
                tc.tile_pool(name="phb", bufs=2) as bpool,
                tc.tile_pool(name="fjp", bufs=3) as fjpool,
                tc.tile_pool(name="psC", bufs=2, space="PSUM") as pscp,
                tc.tile_pool(name="ps2", bufs=2, space="PSUM") as ps2p,
                tc.tile_pool(name="ps3", bufs=1, space="PSUM") as ps3p,
            ):
                state = {}
                psc_box = [None]

                def issue_front(bt):
                    # w on a bufs=1 pool: batch bt's load issues only after
                    # batch bt-1's scatter released the buffer, keeping the
                    # lead-in DMA queues clear for phase A + the f_lo writes.
                    w_t = wpool.tile([128, kb_fix, F], bf, tag="w")
                    nc.scalar.dma_start(
                        w_t[:], w_ed_e[:, bt * kb_fix:(bt + 1) * kb_fix, :])
                    base8 = bt * kb_fix * 8
                    fj_t = fjpool.tile([128, kb_fix, F], bf, tag="fj")
                    nc.gpsimd.dma_gather(
                        fj_t[:, 0:kb_lo, :], f_lo_hbm[:, :],
                        idx_t[:, base8:base8 + kb_lo * 8],
                        num_idxs=kb_lo * 128,
                        num_idxs_reg=kb_lo * 128,
                        elem_size=F,
                        single_packet=False,
                        queue_num=(2 * bt if k_hi else bt) % 4,
                    )
                    state[bt] = (w_t, fj_t)

                def issue_back(bt):
                    w_t, fj_t = state.pop(bt)
                    # scatter one-hots generated on DVE (saves 10.8MB of DMA
                    # that would otherwise compete with the gather drains);
                    # emitted here so the DVE FIFO orders them after the
                    # phase-A casts: S[p, g, c] = (rel[p, g] == c)
                    s_t = bpool.tile([128, kb_fix, WIN], mybir.dt.float8e4,
                                     tag="s")
                    nc.vector.tensor_tensor(
                        s_t[:],
                        rel_t[:, bt * kb_fix:(bt + 1) * kb_fix, :]
                        .to_broadcast([128, kb_fix, WIN]),
                        iota_t[:].to_broadcast([128, kb_fix, WIN]),
                        mybir.AluOpType.is_equal)
                    base8 = bt * kb_fix * 8
                    if k_hi:
                        nc.gpsimd.dma_gather(
                            fj_t[:, kb_lo:kb_fix, :], f_hi_hbm[:, :],
                            idx_t[:, base8 + kb_lo * 8:base8 + kb_fix * 8],
                            num_idxs=kb_hi * 128,
                            num_idxs_reg=kb_hi * 128,
                            elem_size=F,
                            single_packet=False,
                            queue_num=(2 * bt + 1) % 4,
                        )
                    wf_t = bpool.tile([128, kb_fix, F], bf, tag="wf")
                    nc.vector.tensor_tensor(
                        wf_t[:], w_t[:], fj_t[:], mybir.AluOpType.mult)

                    for wi in range(WB):
                        wk = bt * WB + wi
                        ch = wk // 4
                        col = WIN * (wk % 4)
                        if wk % 4 == 0:
                            psc = pscp.tile([128, CHUNK], f32)
                            psc_box[0] = psc
                        psc = psc_box[0]
                        for g in range(k_fix):
                            sl = (wi * k_lo + g if g < k_lo
                                  else kb_lo + wi * k_hi + (g - k_lo))
                            nc.tensor.matmul(
                                psc[:, col:col + WIN],
                                wf_t[:, sl, :],
                                s_t[:, sl, :],
                                start=(g == 0), stop=(g == k_fix - 1),
                            )

                        if wk % 4 == 3:
                            convT = bpool.tile([128, CHUNK], bf, tag="convT")
                            nc.vector.tensor_copy(convT[:], psc[:])
                            ps2 = ps2p.tile([128, CHUNK], f32)
                            nc.tensor.matmul(ps2[:], w_out_t[:], convT[:],
                                             start=True, stop=True)
                            outT = bpool.tile([128, CHUNK], bf, tag="outT")
                            nc.scalar.activation(
                                outT[:], ps2[:],
                                mybir.ActivationFunctionType.Identity,
                                bias=b_t[:],
                            )
                            outf = bpool.tile([128, 4, F], f32, tag="outf")
                            for t in range(4):
                                ps3 = ps3p.tile([128, 128], bf)
                                nc.tensor.transpose(
                                    ps3[:], outT[:, t * 128:(t + 1) * 128],
                                    id_t[:])
                                nc.vector.tensor_copy(outf[:, t, :], ps3[:])
                            dst = out_e[ch * CHUNK:(ch + 1) * CHUNK,
                                        :].rearrange("(t p) f -> p t f", p=128)
                            nc.sync.dma_start(dst, outf[:])

                # lo-gathers lead the hi-gathers by one batch: the lo calls
                # for batches 0 and 1 fill the window where f_hi is still
                # being written by phase A.
                issue_front(0)
                for bt in range(1, NB):
                    issue_front(bt)
                    issue_back(bt - 1)
                issue_back(NB - 1)

    nc.compile()
    _BUILD_CACHE[key] = nc
    return nc


def _prep(x, w_ij, seg_i, idx_j, W_in, W_out, b_out):
    """Host-side sharding: reorder/pad edges, build S one-hots, wrap idxs."""
    x = np.asarray(x, dtype=np.float32)
    w_ij = np.asarray(w_ij, dtype=np.float32)
    seg = np.asarray(seg_i).astype(np.int64)
    idxj = np.asarray(idx_j).astype(np.int64)

    # Relabel atoms so every 128-atom window gets a near-equal edge count
    # (snake-deal atoms in decreasing edge-count order over the windows).
    # This minimizes the uniform per-window padding k_lo/k_hi, which sets
    # the dma_gather descriptor-generation floor.  seg/idx/x/f-table/output
    # all permute consistently; the output is un-permuted on the host.
    cnt = np.bincount(seg, minlength=N_ATOMS)
    order = np.argsort(-cnt, kind="stable")
    i = np.arange(N_ATOMS)
    r, c = np.divmod(i, N_WIN)
    w = np.where(r % 2 == 0, c, N_WIN - 1 - c)
    perm = np.empty(N_ATOMS, np.int64)
    perm[order] = w * WIN + r
    seg = perm[seg]
    idxj = perm[idxj]
    o = np.argsort(seg, kind="stable")
    seg, idxj, w_ij = seg[o], idxj[o], w_ij[o]

    bounds = np.searchsorted(seg, np.arange(N_WIN + 1) * WIN)
    Hs = A_PAD // 2

    # per-window lo/hi split (f-table halves)
    lo_ids, hi_ids, lo_v, hi_v = [], [], [], []
    n_lo = np.zeros(N_WIN, np.int64)
    n_hi = np.zeros(N_WIN, np.int64)
    for k in range(N_WIN):
        b0, b1 = bounds[k], bounds[k + 1]
        ids = np.arange(b0, b1)
        v = idxj[b0:b1]
        m = v < Hs
        lo_ids.append(ids[m])
        hi_ids.append(ids[~m])
        lo_v.append(v[m].astype(np.int16))
        hi_v.append((v[~m] - Hs).astype(np.int16))
        n_lo[k] = m.sum()
        n_hi[k] = (~m).sum()
    k_lo = max(1, int(np.ceil(n_lo.max() / 128)))
    k_hi = max(1, int(np.ceil(n_hi.max() / 128)))
    k_fix = k_lo + k_hi
    e_win = k_fix * 128
    g_core = WIN_PER_CORE * k_fix
    e_pad = g_core * 128

    # padded edge-id + gather-idx matrices in lo|hi order
    eidx = np.zeros((N_WIN, e_win), np.int64)
    valid = np.zeros((N_WIN, e_win), bool)
    gidx = np.zeros((N_WIN, e_win), np.int16)
    for k in range(N_WIN):
        a, b = n_lo[k], n_hi[k]
        eidx[k, :a] = lo_ids[k]
        valid[k, :a] = True
        gidx[k, :a] = lo_v[k]
        off = k_lo * 128
        eidx[k, off:off + b] = hi_ids[k]
        valid[k, off:off + b] = True
        gidx[k, off:off + b] = hi_v[k]

    w_bf = w_ij.astype(BF16)

    xT = np.zeros((128, A_PAD), BF16)
    xT[:, perm] = np.ascontiguousarray(x.T).astype(BF16)
    shared = {
        "xT": xT,
        "w_in": np.asarray(W_in, np.float32).astype(BF16),
        "w_out": np.asarray(W_out, np.float32).astype(BF16),
        "b_out": np.asarray(b_out, np.float32).reshape(128, 1).copy(),
        "ident": np.eye(128, dtype=BF16),
        "iotaf": np.broadcast_to(
            np.arange(WIN, dtype=np.float32), (128, WIN)).astype(BF16),
    }

    # Slot-block order per core: per WB-window batch, all lo sections then
    # all hi sections (must match the device-side gather/scatter layout).
    n_lo_s, n_hi_s = k_lo * 128, k_hi * 128
    in_maps = []
    for c in range(N_CORES):
        w0 = c * WIN_PER_CORE
        ei_b, va_b, gi_lo, gi_hi, wbase = [], [], [], [], []
        for bt in range(WIN_PER_CORE // WB):
            for wi in range(WB):
                k = w0 + bt * WB + wi
                ei_b.append(eidx[k, :n_lo_s])
                va_b.append(valid[k, :n_lo_s])
                gi_lo.append(gidx[k, :n_lo_s])
                wbase.append(np.full(n_lo_s, k * WIN, np.int64))
            for wi in range(WB):
                k = w0 + bt * WB + wi
                ei_b.append(eidx[k, n_lo_s:])
                va_b.append(valid[k, n_lo_s:])
                gi_hi.append(gidx[k, n_lo_s:])
                wbase.append(np.full(n_hi_s, k * WIN, np.int64))
        ei = np.concatenate(ei_b)
        va = np.concatenate(va_b)
        wb_s = np.concatenate(wbase)

        w_rows = np.zeros((e_pad, F), BF16)
        w_rows[va] = w_bf[ei[va]]
        w_ed = np.ascontiguousarray(
            w_rows.reshape(g_core, 128, F).transpose(1, 0, 2))

        rel = np.where(va, seg[ei] - wb_s, -1)
        relw = np.ascontiguousarray(
            rel.reshape(g_core, 128).T.astype(BF16))

        # wrapped idx layout, one wrap per gather call (lo and hi per batch)
        blocks = []
        for bt in range(WIN_PER_CORE // WB):
            lo_cat = np.concatenate(gi_lo[bt * WB:(bt + 1) * WB])
            blocks.append(lo_cat.reshape(-1, 16).T)
            if k_hi:
                hi_cat = np.concatenate(gi_hi[bt * WB:(bt + 1) * WB])
                blocks.append(hi_cat.reshape(-1, 16).T)
        idxw = np.ascontiguousarray(
            np.tile(np.concatenate(blocks, axis=1), (8, 1)))

        m = dict(shared)
        m["w_ed"] = w_ed
        m["relw"] = relw
        m["idxw"] = idxw
        in_maps.append(m)
    return k_lo, k_hi, in_maps, perm


def kernel(x, w_ij, seg_i, idx_j, seg_i_sum, W_in, W_out, b_out):
    k_lo, k_hi, in_maps, perm = _prep(x, w_ij, seg_i, idx_j, W_in, W_out,
                                      b_out)
    nc = _build(k_lo, k_hi)
    res = run_bass_kernel_spmd(nc, in_maps, core_ids=list(range(N_CORES)),
                               trace=TRACE)
    kernel.last_result = res
    out = np.concatenate(
        [np.asarray(res.results[c]["out"]) for c in range(N_CORES)], axis=0)
    return np.ascontiguousarray(out[perm]).astype(np.float32)



# revision 37
# speedup vs baseline: 1.1480x; 1.0016x over previous
"""CFConv (SchNet continuous-filter convolution) on 8 TRN2 NeuronCores.

Reference computation:
    f    = x @ W_in                       # (20000, 128)
    f_j  = f[idx_j]                       # (640000, 128) gather
    wf   = w_ij * f_j                     # elementwise
    conv = segment_sum(wf, seg_i)         # (20000, 128), seg_i sorted
    out  = conv @ W_out + b_out

Distribution: seg_i is sorted, so atoms are split into 8 contiguous
ranges of 2560 (padded to 20480); each core gets the edges targeting its
atom range.  No collectives needed — each core owns its output rows.

Per-core device pipeline (matmuls bf16 x bf16 or bf16 x fp8, f32 PSUM):
  Phase A: f = x @ W_in computed locally (replicated), written to two
           internal HBM half-tables (bf16 rows, split so phase B can
           start after only the low half is ready).  xT streams in 8
           small chunks (pipeline starts on the first 2560 atoms) and
           the psum->bf16 casts alternate between DVE and ACT so
           neither engine's other work stalls the f-table chain.
  Phase B: edges processed in groups of 128 (one group = one matmul
           contraction), host-packed per 128-atom window, batched WB=2
           windows per gather-call pair.  A dma_gather holds the Pool
           engine through its DMA drain (~2.4-2.8ns/idx observed), so
           the 20 calls are the serial throughput floor; everything
           else is arranged to keep that stream gapless:
    - lo-gathers lead hi-gathers by one batch (f_hi still writing)
    - idx DMA issues first on the scalar HWDGE queue, then the xT
      chunks; w tiles use a bufs=1 pool so only one batch prefetches,
      keeping lead-in bandwidth for the f-table writes the first
      gather waits on
    - scatter one-hots are generated on DVE (iota==rel broadcast
      is_equal -> fp8), saving 10.8MB/core of DMA; their rel input is
      DMA'd after the xT chunks so the IS_EQ ops cannot be scheduled
      into phase A
    - wf = w * f_j on VectorE
    - segment-sum via TensorE: psum[fm, atom_window] += wf_g^T @ S_g
    - out^T = W_out^T @ conv^T (TensorE), bias via ScalarE, transposed
      back per 128x128 tile on TensorE, DMA'd to the output shard.

Atoms are host-relabeled (snake-deal by per-atom edge count) so every
window carries a near-equal edge count, minimizing the uniform padding;
the output is un-permuted on the host after the gather.

Found not to work: prepare_only+trigger_dma (SWDGE ring blocks, Tile
consumer sync broken), indirect_dma_start (7x slower + wrong data on
HW), single merged f table (a call drains through one SWDGE queue's
engines only - the lo/hi pair's 2-queue parallelism is load-bearing),
-1 padding indices (hangs the DMA completion sem accounting).

Measured on 8 axon TRN2 cores: ~332-343 us HW exec (vs ~399 us for
the 40-call baseline on the same session; run-to-run HBM-neighbor
noise is +-15%), rel err 5.2e-3 vs the f32 reference.
"""

import numpy as np
import ml_dtypes

import concourse.bacc as bacc
import concourse.bass as bass
import concourse.mybir as mybir
import concourse.tile as tile
from concourse.bass_utils import run_bass_kernel_spmd

BF16 = ml_dtypes.bfloat16
FP8 = ml_dtypes.float8_e4m3

N_ATOMS = 20000
N_EDGES = 640000
F = 128
N_CORES = 8
A_CORE = 2560                 # padded atoms per core
A_PAD = A_CORE * N_CORES      # 20480
CHUNK = 512                   # atoms per PSUM chunk (one bank)
WIN = 128                     # atoms per window (matmul N dim)
WIN_PER_CORE = A_CORE // WIN  # 20
N_WIN = A_PAD // WIN          # 160
WB = 2                        # windows per gather-call batch

TRACE = False                 # set True (with ntff shim) for profiling
_BUILD_CACHE: dict = {}


def _build(k_lo: int, k_hi: int):
    """Build the SPMD Bass graph for given per-window group counts.

    Each 128-atom window's edges are split into a lo half (f rows
    [0, A_PAD/2)) and a hi half, each padded to k_lo/k_hi groups of 128;
    the two dma_gathers per window depend only on their half of the f
    table, so phase B overlaps the tail of phase A.
    """
    key = (k_lo, k_hi)
    if key in _BUILD_CACHE:
        return _BUILD_CACHE[key]

    k_fix = k_lo + k_hi
    G = WIN_PER_CORE * k_fix      # groups per core
    E = G * 128                   # padded edges per core
    H = A_PAD if k_hi == 0 else A_PAD // 2
    bf = mybir.dt.bfloat16
    f32 = mybir.dt.float32

    nc = bacc.Bacc("TRN2", target_bir_lowering=False, debug=False,
                   num_swdge_queues=4, num_devices=N_CORES)
    xT_e = nc.dram_tensor("xT", [128, A_PAD], bf, kind="ExternalInput")
    w_in_e = nc.dram_tensor("w_in", [128, 128], bf, kind="ExternalInput")
    w_out_e = nc.dram_tensor("w_out", [128, 128], bf, kind="ExternalInput")
    b_e = nc.dram_tensor("b_out", [128, 1], f32, kind="ExternalInput")
    w_ed_e = nc.dram_tensor("w_ed", [128, G, F], bf, kind="ExternalInput")
    rel_e = nc.dram_tensor("relw", [128, G], bf, kind="ExternalInput")
    iota_e = nc.dram_tensor("iotaf", [128, WIN], bf, kind="ExternalInput")
    idx_e = nc.dram_tensor("idxw", [128, E // 16], mybir.dt.int16,
                           kind="ExternalInput")
    out_e = nc.dram_tensor("out", [F, A_CORE], bf, kind="ExternalOutput")

    with tile.TileContext(nc) as tc:
        with (
            tc.tile_pool(name="dram", bufs=1, space="DRAM") as dpool,
            tc.tile_pool(name="const", bufs=1) as cpool,
            tc.tile_pool(name="pha", bufs=3) as apool,
            tc.tile_pool(name="psA", bufs=3, space="PSUM") as psA,
        ):
            f_lo_hbm = dpool.tile([H, F], bf)
            f_hi_hbm = dpool.tile([H, F], bf, name="f_hi_hbm") if k_hi else None

            w_in_t = cpool.tile([128, 128], bf)
            nc.sync.dma_start(w_in_t[:], w_in_e[:])
            w_out_t = cpool.tile([128, 128], bf)
            nc.sync.dma_start(w_out_t[:], w_out_e[:])
            b_t = cpool.tile([128, 1], f32)
            nc.sync.dma_start(b_t[:], b_e[:])
            iota_t = cpool.tile([128, 1, WIN], bf)
            nc.sync.dma_start(iota_t[:, 0, :], iota_e[:])
            # idx issues first on the scalar HWDGE queue (it gates the
            # first gather); then the xT chunks — 8 small ones so the
            # matmul pipeline starts as soon as the first 2560 atoms land.
            idx_t = cpool.tile([128, E // 16], mybir.dt.int16)
            nc.scalar.dma_start(idx_t[:], idx_e[:])

            # ---------------- Phase A: f table ----------------
            QW = A_PAD // 8
            f_sb = None
            for x4 in range(8):
                xq_t = apool.tile([128, QW], bf, tag="xq")
                nc.scalar.dma_start(xq_t[:], xT_e[:, x4 * QW:(x4 + 1) * QW])
                for t4q in range(QW // 512):
                    t4 = x4 * (QW // 512) + t4q
                    ps = psA.tile([128, 4, 128], f32)
                    for q in range(4):
                        tl = t4q * 4 + q
                        nc.tensor.matmul(
                            ps[:, q, :],
                            xq_t[:, tl * 128:(tl + 1) * 128],
                            w_in_t[:],
                            start=True, stop=True,
                        )
                    j = t4 % 2
                    if j == 0:
                        f_sb = apool.tile([128, 8, F], bf, tag="fsb")
                    # alternate the psum->bf16 casts between DVE and ACT so
                    # neither engine's other work (IS_EQ one-hots on DVE,
                    # DMA issue on ACT) can stall the f-table chain
                    if t4 % 2 == 0:
                        nc.vector.tensor_copy(
                            f_sb[:, j * 4:(j + 1) * 4, :], ps[:])
                    else:
                        nc.scalar.copy(
                            f_sb[:, j * 4:(j + 1) * 4, :], ps[:])
                    if j == 1:
                        a0 = (t4 - 1) * 512
                        tgt = f_lo_hbm if a0 < H else f_hi_hbm
                        a0 = a0 % H
                        dst = tgt[a0:a0 + 1024, :].rearrange(
                            "(j p) f -> p j f", p=128)
                        nc.sync.dma_start(dst, f_sb[:])

            # rel lands only after the xT chunks: its consumers (the IS_EQ
            # one-hot builders on DVE) must not be schedulable during phase
            # A, where they would displace the psum->f casts from the DVE
            # and stretch the f-table critical path.
            rel_t = cpool.tile([128, G, 1], bf)
            nc.scalar.dma_start(rel_t[:, :, 0], rel_e[:])

            # ---------------- Phase B: edges ----------------
            # Gathers batched over WB windows per call pair (lo + hi): slot
            # layout per batch is [w0.lo .. w_{WB-1}.lo | w0.hi .. w_{WB-1}.hi]
            # (host-matched).  Fewer calls amortize the ~2us fixed cost each
            # dma_gather pays on the GpSimd engine.
            kb_lo = WB * k_lo
            kb_hi = WB * k_hi
            kb_fix = kb_lo + kb_hi
            NB = WIN_PER_CORE // WB
            with (
                tc.tile_pool(name="wp", bufs=1) as wpool,
                tc.tile_pool(name="phb", bufs=2) as bpool,
                tc.tile_pool(name="fjp", bufs=3) as fjpool,
                tc.tile_pool(name="psC", bufs=2, space="PSUM") as pscp,
                tc.tile_pool(name="ps2", bufs=2, space="PSUM") as ps2p,
            ):
                state = {}
                psc_box = [None]

                def issue_front(bt):
                    # w on a bufs=1 pool: batch bt's load issues only after
                    # batch bt-1's scatter released the buffer, keeping the
                    # lead-in DMA queues clear for phase A + the f_lo writes.
                    w_t = wpool.tile([128, kb_fix, F], bf, tag="w")
                    nc.scalar.dma_start(
                        w_t[:], w_ed_e[:, bt * kb_fix:(bt + 1) * kb_fix, :])
                    base8 = bt * kb_fix * 8
                    fj_t = fjpool.tile([128, kb_fix, F], bf, tag="fj")
                    nc.gpsimd.dma_gather(
                        fj_t[:, 0:kb_lo, :], f_lo_hbm[:, :],
                        idx_t[:, base8:base8 + kb_lo * 8],
                        num_idxs=kb_lo * 128,
                        num_idxs_reg=kb_lo * 128,
                        elem_size=F,
                        single_packet=False,
                        queue_num=(2 * bt if k_hi else bt) % 4,
                    )
                    state[bt] = (w_t, fj_t)

                def issue_back(bt):
                    w_t, fj_t = state.pop(bt)
                    # scatter one-hots generated on DVE (saves 10.8MB of DMA
                    # that would otherwise compete with the gather drains);
                    # emitted here so the DVE FIFO orders them after the
                    # phase-A casts: S[p, g, c] = (rel[p, g] == c)
                    s_t = bpool.tile([128, kb_fix, WIN], mybir.dt.float8e4,
                                     tag="s")
                    nc.vector.tensor_tensor(
                        s_t[:],
                        rel_t[:, bt * kb_fix:(bt + 1) * kb_fix, :]
                        .to_broadcast([128, kb_fix, WIN]),
                        iota_t[:].to_broadcast([128, kb_fix, WIN]),
                        mybir.AluOpType.is_equal)
                    base8 = bt * kb_fix * 8
                    if k_hi:
                        nc.gpsimd.dma_gather(
                            fj_t[:, kb_lo:kb_fix, :], f_hi_hbm[:, :],
                            idx_t[:, base8 + kb_lo * 8:base8 + kb_fix * 8],
                            num_idxs=kb_hi * 128,
                            num_idxs_reg=kb_hi * 128,
                            elem_size=F,
                            single_packet=False,
                            queue_num=(2 * bt + 1) % 4,
                        )
                    wf_t = bpool.tile([128, kb_fix, F], bf, tag="wf")
                    nc.vector.tensor_tensor(
                        wf_t[:], w_t[:], fj_t[:], mybir.AluOpType.mult)

                    for wi in range(WB):
                        wk = bt * WB + wi
                        ch = wk // 4
                        col = WIN * (wk % 4)
                        if wk % 4 == 0:
                            psc = pscp.tile([128, CHUNK], f32)
                            psc_box[0] = psc
                        psc = psc_box[0]
                        for g in range(k_fix):
                            sl = (wi * k_lo + g if g < k_lo
                                  else kb_lo + wi * k_hi + (g - k_lo))
                            nc.tensor.matmul(
                                psc[:, col:col + WIN],
                                wf_t[:, sl, :],
                                s_t[:, sl, :],
                                start=(g == 0), stop=(g == k_fix - 1),
                            )

                        if wk % 4 == 3:
                            convT = bpool.tile([128, CHUNK], bf, tag="convT")
                            nc.vector.tensor_copy(convT[:], psc[:])
                            ps2 = ps2p.tile([128, CHUNK], f32)
                            nc.tensor.matmul(ps2[:], w_out_t[:], convT[:],
                                             start=True, stop=True)
                            outT = bpool.tile([128, CHUNK], bf, tag="outT")
                            nc.scalar.activation(
                                outT[:], ps2[:],
                                mybir.ActivationFunctionType.Identity,
                                bias=b_t[:],
                            )
                            # output stays transposed [F, atoms]; the host
                            # untransposes for free (outT is already bf16,
                            # so precision is unchanged)
                            nc.sync.dma_start(
                                out_e[:, ch * CHUNK:(ch + 1) * CHUNK],
                                outT[:])

                # lo-gathers lead the hi-gathers by one batch: the lo calls
                # for batches 0 and 1 fill the window where f_hi is still
                # being written by phase A.
                issue_front(0)
                for bt in range(1, NB):
                    issue_front(bt)
                    issue_back(bt - 1)
                issue_back(NB - 1)

    nc.compile()
    _BUILD_CACHE[key] = nc
    return nc


def _prep(x, w_ij, seg_i, idx_j, W_in, W_out, b_out):
    """Host-side sharding: reorder/pad edges, build S one-hots, wrap idxs."""
    x = np.asarray(x, dtype=np.float32)
    w_ij = np.asarray(w_ij, dtype=np.float32)
    seg = np.asarray(seg_i).astype(np.int64)
    idxj = np.asarray(idx_j).astype(np.int64)

    # Relabel atoms so every 128-atom window gets a near-equal edge count
    # (snake-deal atoms in decreasing edge-count order over the windows).
    # This minimizes the uniform per-window padding k_lo/k_hi, which sets
    # the dma_gather descriptor-generation floor.  seg/idx/x/f-table/output
    # all permute consistently; the output is un-permuted on the host.
    cnt = np.bincount(seg, minlength=N_ATOMS)
    order = np.argsort(-cnt, kind="stable")
    i = np.arange(N_ATOMS)
    r, c = np.divmod(i, N_WIN)
    w = np.where(r % 2 == 0, c, N_WIN - 1 - c)
    perm = np.empty(N_ATOMS, np.int64)
    perm[order] = w * WIN + r
    seg = perm[seg]
    idxj = perm[idxj]
    o = np.argsort(seg, kind="stable")
    seg, idxj, w_ij = seg[o], idxj[o], w_ij[o]

    bounds = np.searchsorted(seg, np.arange(N_WIN + 1) * WIN)
    Hs = A_PAD // 2

    # per-window lo/hi split (f-table halves)
    lo_ids, hi_ids, lo_v, hi_v = [], [], [], []
    n_lo = np.zeros(N_WIN, np.int64)
    n_hi = np.zeros(N_WIN, np.int64)
    for k in range(N_WIN):
        b0, b1 = bounds[k], bounds[k + 1]
        ids = np.arange(b0, b1)
        v = idxj[b0:b1]
        m = v < Hs
        lo_ids.append(ids[m])
        hi_ids.append(ids[~m])
        lo_v.append(v[m].astype(np.int16))
        hi_v.append((v[~m] - Hs).astype(np.int16))
        n_lo[k] = m.sum()
        n_hi[k] = (~m).sum()
    k_lo = max(1, int(np.ceil(n_lo.max() / 128)))
    k_hi = max(1, int(np.ceil(n_hi.max() / 128)))
    k_fix = k_lo + k_hi
    e_win = k_fix * 128
    g_core = WIN_PER_CORE * k_fix
    e_pad = g_core * 128

    # padded edge-id + gather-idx matrices in lo|hi order
    eidx = np.zeros((N_WIN, e_win), np.int64)
    valid = np.zeros((N_WIN, e_win), bool)
    gidx = np.zeros((N_WIN, e_win), np.int16)
    for k in range(N_WIN):
        a, b = n_lo[k], n_hi[k]
        eidx[k, :a] = lo_ids[k]
        valid[k, :a] = True
        gidx[k, :a] = lo_v[k]
        off = k_lo * 128
        eidx[k, off:off + b] = hi_ids[k]
        valid[k, off:off + b] = True
        gidx[k, off:off + b] = hi_v[k]

    w_bf = w_ij.astype(BF16)

    xT = np.zeros((128, A_PAD), BF16)
    xT[:, perm] = np.ascontiguousarray(x.T).astype(BF16)
    shared = {
        "xT": xT,
        "w_in": np.asarray(W_in, np.float32).astype(BF16),
        "w_out": np.asarray(W_out, np.float32).astype(BF16),
        "b_out": np.asarray(b_out, np.float32).reshape(128, 1).copy(),
        "iotaf": np.broadcast_to(
            np.arange(WIN, dtype=np.float32), (128, WIN)).astype(BF16),
    }

    # Slot-block order per core: per WB-window batch, all lo sections then
    # all hi sections (must match the device-side gather/scatter layout).
    n_lo_s, n_hi_s = k_lo * 128, k_hi * 128
    in_maps = []
    for c in range(N_CORES):
        w0 = c * WIN_PER_CORE
        ei_b, va_b, gi_lo, gi_hi, wbase = [], [], [], [], []
        for bt in range(WIN_PER_CORE // WB):
            for wi in range(WB):
                k = w0 + bt * WB + wi
                ei_b.append(eidx[k, :n_lo_s])
                va_b.append(valid[k, :n_lo_s])
                gi_lo.append(gidx[k, :n_lo_s])
                wbase.append(np.full(n_lo_s, k * WIN, np.int64))
            for wi in range(WB):
                k = w0 + bt * WB + wi
                ei_b.append(eidx[k, n_lo_s:])
                va_b.append(valid[k, n_lo_s:])
                gi_hi.append(gidx[k, n_lo_s:])
                wbase.append(np.full(n_hi_s, k * WIN, np.int64))
        ei = np.concatenate(ei_b)
        va = np.concatenate(va_b)
        wb_s = np.concatenate(wbase)

        w_rows = np.zeros((e_pad, F), BF16)
        w_rows[va] = w_bf[ei[va]]
        w_ed = np.ascontiguousarray(
            w_rows.reshape(g_core, 128, F).transpose(1, 0, 2))

        rel = np.where(va, seg[ei] - wb_s, -1)
        relw = np.ascontiguousarray(
            rel.reshape(g_core, 128).T.astype(BF16))

        # wrapped idx layout, one wrap per gather call (lo and hi per batch)
        blocks = []
        for bt in range(WIN_PER_CORE // WB):
            lo_cat = np.concatenate(gi_lo[bt * WB:(bt + 1) * WB])
            blocks.append(lo_cat.reshape(-1, 16).T)
            if k_hi:
                hi_cat = np.concatenate(gi_hi[bt * WB:(bt + 1) * WB])
                blocks.append(hi_cat.reshape(-1, 16).T)
        idxw = np.ascontiguousarray(
            np.tile(np.concatenate(blocks, axis=1), (8, 1)))

        m = dict(shared)
        m["w_ed"] = w_ed
        m["relw"] = relw
        m["idxw"] = idxw
        in_maps.append(m)
    return k_lo, k_hi, in_maps, perm


def kernel(x, w_ij, seg_i, idx_j, seg_i_sum, W_in, W_out, b_out):
    k_lo, k_hi, in_maps, perm = _prep(x, w_ij, seg_i, idx_j, W_in, W_out,
                                      b_out)
    nc = _build(k_lo, k_hi)
    res = run_bass_kernel_spmd(nc, in_maps, core_ids=list(range(N_CORES)),
                               trace=TRACE)
    kernel.last_result = res
    out = np.concatenate(
        [np.asarray(res.results[c]["out"]).T.astype(np.float32)
         for c in range(N_CORES)], axis=0)
    return np.ascontiguousarray(out[perm])

